# revision 22
# baseline (speedup 1.0000x reference)
"""Trainium2 Bass kernel for multi-head self-attention with Q=K=V=x@Wq.

Problem: x [4, 2048, 512] f32, Wq [512, 512] f32, HEAD=8 (head_dim=64).
  q = x @ Wq;  per (b, h): S = q_h q_h^T / 8; out = softmax(S) @ q_h.

Sharding (8 cores): core i -> batch b = i//2, head group g = i%2 (4 heads).
Each core gets x[b]^T (bf16, host-transposed) and its Wq column slice;
produces out[b, :, 256g:256g+256].  No cross-core communication.

On-core algorithm (v6):
  Startup: q_T [128, 2, 2048] bf16 computed DIRECTLY as wq^T x^T with the
    small wq slices stationary (kb-outer so each accumulator's DVE copy
    overlaps the next kb's matmuls), then q_nat [128, 16, 4, 65] bf16 (ctx
    stationary, 65th col ones for the free Z row) via 32 PE transposes of
    q_T.  No q_pair, no ScalarE copies; bf16 inputs halve the DMA-in.
  Main loop per (pair p, query column qc of 512), 16 key blocks jb:
    - drain ctx(jb-3) immediately before S(jb) so both PE semaphore waits
      are pre-satisfied at dispatch (exp is >1 jb ahead).
    - S pair: two K=64 matmuls, tile_position rows (0,0)/(64,0) derived
      from base_partition, running concurrently (measured 1.89x).
    - exp split: jb even -> DVE Schraudolph bit-trick (int16(A*s+B)
      reinterpreted as bf16, C=7.4 mean-unbiased so mixing with exact
      blocks is softmax-neutral; 1226ns measured); jb odd -> ScalarE
      exact ACT exp (1147ns).  The steady state is limited by the
      S->sem->exp->sem->S(+2) latency chain (~1740ns/2jb); both engines
      run within it.
    - ctx: 2 accumulating N=512 matmuls into cpA/cpB [65, 512].
  Tail per (p, qc): cp->csb copies split into 4 half-copies, transposes
  into batched [128,4,65] tiles (tpp bufs=2), reciprocal, per-piece muls
  -- all sized under the per-jb slack of each engine and alternated
  DVE/ScalarE so no exp (and hence no S matmul) is delayed; DMA out
  alternates sync/gpsimd; drained 1-2 pieces/jb during the next sweep.
  PSUM banks: S pairs 2x2 + cpA 1 + cpB 1 + tpp 2 = 8 (exactly).
  HAM: warmup dummies into tpp slots; startup is dense PE work; final
  drain dummies into the (free) ps pool.
  NOTE: do NOT pre-slice bitcast APs when queueing pending ctx work --
  storing sliced bitcast APs across the emission/drain boundary broke
  Tile dependency tracking (nondeterministic corruption); slice at drain.
"""

import sys

sys.path.insert(0, "/opt/trn_rl_repo")

from contextlib import ExitStack

import numpy as np

import concourse.bass as bass
import concourse.tile as tile
from concourse import bacc, mybir
from concourse.masks import make_identity

B, S, D, HEAD = 4, 2048, 512, 8
HD = D // HEAD  # 64
EC = 256  # e-columns per core (4 heads)
F32 = mybir.dt.float32
F32R = mybir.dt.float32r
BF16 = mybir.dt.bfloat16
I16 = mybir.dt.int16
N_CORES = 8

# Schraudolph exp(s/8) in bf16-bit space: bits = A*s + B (C=7.4 calibrated
# mean-unbiased vs exact exp over the score distribution)
SCH_A = 128.0 / float(np.log(2.0)) / 8.0
SCH_B = 16256.0 - 7.4

_PROGRAM = None


def build_program():
    nc = bacc.Bacc(None, target_bir_lowering=False)
    # x^T bf16: xt[p, dc, kb, i] = x[kb*512 + i, dc*128 + p]
    xt_d = nc.dram_tensor("xt", [128, 4 * 4 * 512], BF16, kind="ExternalInput")
    # wq bf16: wq[p, dc, e] = Wq[dc*128 + p, g*256 + e]
    wq_d = nc.dram_tensor("wq", [128, 4 * EC], BF16, kind="ExternalInput")
    out_d = nc.dram_tensor("out", [S, EC], F32, kind="ExternalOutput")

    xt_r = xt_d.rearrange("p (dc kb i) -> dc p kb i", dc=4, kb=4)
    wq_r = wq_d.rearrange("p (dc e) -> p dc e", dc=4)
    out_r = out_d.rearrange("(ib p) e -> ib p e", p=128)  # [16, 128, 256]

    with tile.TileContext(nc) as tc, ExitStack() as ctx:
        sb = ctx.enter_context(tc.tile_pool(name="sb", bufs=1))
        ep = ctx.enter_context(tc.tile_pool(name="ep", bufs=6))
        csbp = ctx.enter_context(tc.tile_pool(name="csbp", bufs=4))
        ob = ctx.enter_context(tc.tile_pool(name="ob", bufs=6))
        rzp = ctx.enter_context(tc.tile_pool(name="rzp", bufs=6))
        ps = ctx.enter_context(tc.tile_pool(name="ps", bufs=2, space="PSUM"))
        cpa = ctx.enter_context(tc.tile_pool(name="cpa", bufs=1, space="PSUM"))
        cpb = ctx.enter_context(tc.tile_pool(name="cpb", bufs=1, space="PSUM"))
        tpp = ctx.enter_context(tc.tile_pool(name="tpp", bufs=2, space="PSUM"))

        # Warmup / HAM ignition: dep-free matmuls into tpp slots (tail tiles
        # aren't live until the first sweep ends).
        wrm = sb.tile([128, 512], F32R)
        nc.vector.memset(wrm.bitcast(F32), 0.0)

        def dummy_mm():
            wf = tpp.tile([65, 512], F32, tag="tp")
            nc.tensor.matmul(wf, wrm[:, 0:65], wrm, start=True, stop=True)

        def dummy_mm_ps():
            t = ps.tile([128, 1024], F32, tag="ps")
            nc.tensor.matmul(
                t[0:65, 0:512], wrm[:, 0:65], wrm, start=True, stop=True
            )

        for i in range(8):
            dummy_mm()
        # one-time exp table load while ACT is idle
        dume = sb.tile([128, 1], F32)
        nc.scalar.activation(
            dume, wrm[:, 0:1].bitcast(F32), mybir.ActivationFunctionType.Exp
        )

        ident = sb.tile([128, 128], F32)
        make_identity(nc, ident)
        ident_b = sb.tile([128, 128], BF16)
        nc.vector.tensor_copy(ident_b, ident)

        wq_sb = sb.tile([128, 4, EC], BF16)
        nc.sync.dma_start(out=wq_sb, in_=wq_r)
        xt = sb.tile([128, 4, 4, 512], BF16)  # [dcp, dc, kb, i]
        for dc in range(4):
            nc.sync.dma_start(out=xt[:, dc, :, :], in_=xt_r[dc])

        q_T = sb.tile([128, 2, S], BF16)  # [sub*64+e, p, i]
        q_nat = sb.tile([128, 16, 4, 65], BF16)  # [j_in_block, jb, h, e|1]
        nc.vector.memset(q_nat[:, :, :, 64:65], 1.0)

        # ---- Startup: q_T = wq^T @ x^T directly (wq stationary) ----
        # cover the DMA wait with dep-free dummies, then the stream is dense
        for _ in range(4):
            dummy_mm()
        # kb-outer so each [128,512] accumulator's DVE copy overlaps the
        # next kb's matmuls (2-deep ps rotation)
        for p in range(2):
            for kb in range(4):
                t = ps.tile([128, 512], F32, tag="ps")
                for dc in range(4):
                    nc.tensor.matmul(
                        t,
                        wq_sb[:, dc, p * 128 : p * 128 + 128],
                        xt[:, dc, kb, :],
                        start=(dc == 0),
                        stop=(dc == 3),
                    )
                nc.vector.tensor_copy(q_T[:, p, kb * 512 : (kb + 1) * 512], t)
        # q_nat via PE transposes of q_T (4 jb per batched psum tile)
        for p in range(2):
            for jg in range(4):
                tb = tpp.tile([128, 4, 128], BF16, tag="tp")
                for j in range(4):
                    jb = jg * 4 + j
                    nc.tensor.transpose(
                        tb[:, j, :],
                        q_T[:, p, jb * 128 : (jb + 1) * 128],
                        ident_b,
                    )
                nc.vector.tensor_copy(
                    q_nat[:, jg * 4 : (jg + 1) * 4, 2 * p : 2 * p + 2, 0:64],
                    tb.rearrange("j jb (h e) -> j jb h e", h=2),
                )

        # bridge: keep PE busy while the q_nat DVE copies finish
        for i in range(4):
            dummy_mm()

        # ---- Main loop + interleaved tails ----
        pending_tail = []

        def make_tail(p, qc, cA, cB):
            # pieces sized under the per-jb slack of each engine (DVE ~520ns,
            # Scalar ~600ns) and strictly alternated so no exp is delayed
            csA = csbp.tile([65, 512], F32, tag="csA")
            csB = csbp.tile([65, 512], F32, tag="csB")
            pieces = [
                lambda: nc.vector.tensor_copy(csA[:, 0:256], cA[:, 0:256]),
                lambda: nc.scalar.copy(csA[:, 256:512], cA[:, 256:512]),
                lambda: nc.vector.tensor_copy(csB[:, 0:256], cB[:, 0:256]),
                lambda: nc.scalar.copy(csB[:, 256:512], cB[:, 256:512]),
            ]

            def make_head(side, csb):
                h = 2 * p + side
                tp = tpp.tile([128, 4, 65], F32, tag="tp")

                def trans():
                    for j in range(4):
                        nc.tensor.transpose(
                            tp[:, j, :],
                            csb[:, j * 128 : (j + 1) * 128],
                            ident[0:65, 0:65],
                        )

                rz = rzp.tile([128, 4], F32, tag="rz")

                def recip():
                    nc.vector.reciprocal(rz, tp[:, :, 64:65])

                def make_mul(j):
                    def mul():
                        ot = ob.tile([128, 64], F32, tag="ot")
                        if j % 2:
                            nc.scalar.mul(ot, tp[:, j, 0:64], rz[:, j : j + 1])
                        else:
                            nc.vector.tensor_scalar_mul(
                                ot, tp[:, j, 0:64], rz[:, j : j + 1]
                            )
                        eng = nc.gpsimd if j % 2 else nc.sync
                        eng.dma_start(
                            out=out_r[qc * 4 + j, :, h * 64 : (h + 1) * 64],
                            in_=ot,
                        )

                    return mul

                return [trans, recip] + [make_mul(j) for j in range(4)]

            pieces += make_head(0, csA)
            pieces += make_head(1, csB)
            return pieces

        pending = []  # (jb, eb_ap, cA, cB, p, tail_args)

        def drain_one_ctx():
            nonlocal pending_tail
            jb, ebap, cA, cB, p, last = pending.pop(0)
            nc.tensor.matmul(
                cA,
                q_nat[:, jb, 2 * p, 0:65],
                ebap[:, 0:512],
                start=(jb == 0),
                stop=(jb == 15),
            )
            nc.tensor.matmul(
                cB,
                q_nat[:, jb, 2 * p + 1, 0:65],
                ebap[:, 512:1024],
                start=(jb == 0),
                stop=(jb == 15),
            )
            if jb == 15:
                assert not pending_tail
                pending_tail = make_tail(*last)

        for p in range(2):
            qT0 = q_T[0:64, p, :]
            qT1 = q_T[64:128, p, :]
            for qc in range(4):
                cA = cpa.tile([65, 512], F32, tag="cA")
                cB = cpb.tile([65, 512], F32, tag="cB")
                qcs = slice(qc * 512, (qc + 1) * 512)
                for jb in range(16):
                    # ctx(jb-3) immediately before S(jb): both PE waits are
                    # pre-satisfied at dispatch (exp is >1 jb ahead)
                    while len(pending) > 3:
                        drain_one_ctx()
                    jbs = slice(jb * 128, (jb + 1) * 128)
                    psp = ps.tile([128, 1024], F32, tag="ps")
                    nc.tensor.matmul(
                        psp[:, 0:512], qT0[:, jbs], qT0[:, qcs],
                        start=True, stop=True,
                    )
                    nc.tensor.matmul(
                        psp[:, 512:1024], qT1[:, jbs], qT1[:, qcs],
                        start=True, stop=True,
                    )
                    if jb % 2 == 0:
                        ebi = ep.tile([128, 1024], I16, tag="eb")
                        nc.vector.tensor_scalar(
                            ebi,
                            psp,
                            SCH_A,
                            SCH_B,
                            mybir.AluOpType.mult,
                            mybir.AluOpType.add,
                        )
                        ebap = ebi.bitcast(BF16)
                    else:
                        ebb = ep.tile([128, 1024], BF16, tag="eb")
                        nc.scalar.activation(
                            ebb, psp, mybir.ActivationFunctionType.Exp,
                            scale=0.125,
                        )
                        ebap = ebb
                    pending.append((jb, ebap, cA, cB, p, (p, qc, cA, cB)))
                    if pending_tail:
                        pending_tail.pop(0)()
                        if len(pending_tail) > 8:
                            pending_tail.pop(0)()
        while pending:
            drain_one_ctx()

        # Final tail drain with a ps-pool dummy bridge over the csb copies.
        for fn in pending_tail[0:4]:
            fn()
        for _ in range(6):
            dummy_mm_ps()
        for fn in pending_tail[4:]:
            fn()

    nc.compile()
    return nc


def get_program():
    global _PROGRAM
    if _PROGRAM is None:
        _PROGRAM = build_program()
    return _PROGRAM


def make_in_maps(x, Wq):
    import ml_dtypes

    bf = ml_dtypes.bfloat16
    x = np.asarray(x, dtype=np.float32)
    Wq = np.asarray(Wq, dtype=np.float32)
    in_maps = []
    for core in range(N_CORES):
        b, g = core // 2, core % 2
        # xt[p, dc, kb, i] = x[b][kb*512 + i, dc*128 + p]
        xt = np.ascontiguousarray(
            x[b].reshape(4, 512, 4, 128).transpose(3, 2, 0, 1).reshape(128, -1)
        ).astype(bf)
        # wq[p, dc, e] = Wq[dc*128 + p, g*256 + e]
        wq = np.ascontiguousarray(
            Wq[:, g * EC : (g + 1) * EC].reshape(4, 128, EC).transpose(1, 0, 2)
            .reshape(128, -1)
        ).astype(bf)
        in_maps.append({"xt": xt, "wq": wq})
    return in_maps


def assemble(results):
    out = np.empty((B, S, D), dtype=np.float32)
    for core in range(N_CORES):
        b, g = core // 2, core % 2
        out[b, :, g * EC : (g + 1) * EC] = results[core]["out"]
    return out


def kernel(x, Wq):
    from concourse.bass_utils import run_bass_kernel_spmd

    nc = get_program()
    res = run_bass_kernel_spmd(nc, make_in_maps(x, Wq), list(range(N_CORES)))
    return assemble(res.results)
